# revision 9
# baseline (speedup 1.0000x reference)
"""Multi-head attention Trainium2 kernel (Bass/Tile, SPMD over 8 cores).

fp16 compute variant: matmul operands in fp16 (2-byte stream rate = 2x fp32),
fp32 PSUM accumulation, fp32 normalization. Rel err vs fp32 reference ~1e-3.

Sharding: data parallel over batch. Core i computes batches [2i, 2i+2).

Structure per core:
  - Host pre-transposes x -> xT [d, s] partition-major and weights ->
    [d, h*e]; 4KB-line DMAs, split across both HWDGE queues (sync+scalar).
  - v projections for BOTH batches run first, chunk-major over 6 PSUM
    tiles so the PE ramps with DMA arrival instead of stalling per tile.
  - Job pipeline over (batch, pair): ST(i) -> QK-proj(i+1) -> PV(i); the
    next pair's projections fill the exp() latency between scores and PV.
  - Scores transposed ST[t,s]; exp on ScalarE with bias=-5 (fp16 headroom);
    the two heads of a pair sit at base partitions 0/64 so their K=64
    matmuls row-pack concurrently on the PE.
  - PV with V_aug stationary (ones cols 64:66, zero pad to 80 ->
    denominators ride along); the two half-chains interleave so LDW hides.
  - One XBAR DMA transpose per pair [80, 2x512] -> [128, (h s), 80]; one
    reciprocal per pair; fused scalar_tensor_tensor normalize per s-chunk
    (both heads at once, denominator broadcast via stride-0 AP).
  - Output fp16 on device; host casts to fp32.
"""

import numpy as np

import concourse.bass as bass
import concourse.mybir as mybir
import concourse.tile as tile
from concourse.bass_utils import run_bass_kernel_spmd

B, S, D, H, DH = 16, 512, 1024, 16, 64
N_CORES = 8
B_LOC = B // N_CORES  # 2 batches per core
C = D // 128  # 8 contraction chunks over d
TC = S // 128  # 4 chunks over s/t
AUG = 80  # DH + 2 ones cols + pad to 16-row multiple for the XBAR transpose
F32 = mybir.dt.float32
FP16 = mybir.dt.float16
SCALE = 1.0 / np.sqrt(np.float32(D))
EXP_BIAS = -5.0  # exp(logit-5): keeps P in fp16 range; cancels in normalize
OSCALE = 1.0 / 16.0  # pre-scale before fp16 transpose; cancels in normalize


def legalize_waits(nc, cap=1):
    """This walrus build supports at most `cap` sync-wait commands per
    instruction; hoist excess waits onto preceding same-engine NoOps."""
    n_split = 0
    for f in nc.m.functions:
        for blk in f.blocks:
            new_insts = []
            for inst in blk.instructions:
                si = getattr(inst, "sync_info", None)
                waits = list(si.on_wait) if si is not None and si.on_wait else []
                if len(waits) > cap:
                    keep, rest = waits[:cap], waits[cap:]
                    while rest:
                        chunk, rest = rest[:cap], rest[cap:]
                        nop = mybir.InstNoOp(
                            name=f"I-waitsplit-{nc.next_id()}", ins=[], outs=[]
                        )
                        nop.engine = inst.engine
                        nop.sync_info = mybir.SyncInfo(on_wait=chunk, on_update=[])
                        nc.register_instruction(nop, overwrite=True)
                        new_insts.append(nop)
                        n_split += 1
                    si.on_wait = keep
                new_insts.append(inst)
            blk.instructions[:] = new_insts
    return n_split


def build_program():
    nc = bass.Bass()
    # xt: partition-major per (batch, half): [B_LOC, 2, 128, 4*S] so each DMA
    # moves 4KB contiguous per partition.
    xt_d = nc.declare_dram_parameter("xt", [B_LOC, 2, 128, 4 * S], FP16, isOutput=False)
    wq_d = nc.declare_dram_parameter("wq", [C, 128, D], FP16, isOutput=False)
    wk_d = nc.declare_dram_parameter("wk", [C, 128, D], FP16, isOutput=False)
    wv_d = nc.declare_dram_parameter("wv", [C, 128, D], FP16, isOutput=False)
    out_d = nc.declare_dram_parameter("out", [B_LOC, S, D], FP16, isOutput=True)

    with tile.TileContext(nc) as tc:
        with (
            tc.tile_pool(name="wpool", bufs=1) as wpool,
            tc.tile_pool(name="xpool", bufs=1) as xpool,
            tc.tile_pool(name="vpool", bufs=4) as vpool,
            tc.tile_pool(name="qkpool", bufs=4) as qkpool,
            tc.tile_pool(name="ppool", bufs=10) as ppool,
            tc.tile_pool(name="opool", bufs=8) as opool,
            tc.tile_pool(name="rpool", bufs=4) as rpool,
            tc.tile_pool(name="ovpool", bufs=4) as ovpool,
            tc.tile_pool(name="otpool", bufs=4) as otpool,
            tc.tile_pool(name="psmm", bufs=2, space="PSUM") as psmm,
            tc.tile_pool(name="stp", bufs=2, space="PSUM") as stp,
            tc.tile_pool(name="psout", bufs=2, space="PSUM") as psout,
        ):
            exp_bias = ovpool.tile([128, 1], F32, tag="expbias", bufs=1)
            nc.vector.memset(exp_bias, EXP_BIAS)

            # ---- input DMAs: xt + wq on the scalar HWDGE queue, wv + wk on
            # sync, so the two transfer streams run in parallel ----
            xts = [
                xpool.tile([128, C, S], FP16, tag=f"xt{b}", name=f"xt{b}")
                for b in range(B_LOC)
            ]
            wq_sb = wpool.tile([128, C, D], FP16, tag="wq")
            wk_sb = wpool.tile([128, C, D], FP16, tag="wk")
            wv_sb = wpool.tile([128, C, D], FP16, tag="wv")
            # b0 chunk-at-a-time so the first matmul can start early; b1 in
            # two big 4KB-line transfers
            for c in range(C):
                nc.scalar.dma_start(
                    out=xts[0][:, c, :],
                    in_=xt_d[0, c // 4, :, (c % 4) * S : (c % 4 + 1) * S],
                )
            for hf in range(2):
                nc.scalar.dma_start(
                    out=xts[1][:, 4 * hf : 4 * hf + 4, :].rearrange(
                        "p c s -> p (c s)"
                    ),
                    in_=xt_d[1, hf],
                )
            for c in range(C):
                nc.sync.dma_start(out=wv_sb[:, c, :], in_=wv_d[c])
            for g in range(2):
                nc.scalar.dma_start(
                    out=wq_sb[:, 4 * g : 4 * g + 4, :],
                    in_=wq_d[4 * g : 4 * g + 4].rearrange("c p d -> p c d"),
                )
                nc.sync.dma_start(
                    out=wk_sb[:, 4 * g : 4 * g + 4, :],
                    in_=wk_d[4 * g : 4 * g + 4].rearrange("c p d -> p c d"),
                )

            # ---- v projections for BOTH batches, chunk-major so each
            # arriving (xt, wv) chunk unlocks a burst of matmuls ----
            # V_aug layout [128(t), h, 64(e) + ones(2) + zero pad(14)]
            vaugs = {}
            for b in range(B_LOC):
                vaugs[b] = [
                    vpool.tile(
                        [128, H, AUG], FP16, tag=f"vaug{b}", name=f"vaug{b}_{t}"
                    )
                    for t in range(TC)
                ]
                for t in range(TC):
                    nc.vector.memset(vaugs[b][t][:, :, DH : DH + 2], 1.0)
                    nc.vector.memset(vaugs[b][t][:, :, DH + 2 : AUG], 0.0)
            for b in range(B_LOC):
                jobs = [(t, half) for t in range(TC) for half in range(2)]
                # groups of 6 and 2, borrowing stp/psout slots during this phase
                for group in (jobs[:6], jobs[6:]):
                    tiles = {}
                    for gi, (t, half) in enumerate(group):
                        pool = (psmm, psmm, stp, stp, psout, psout)[gi]
                        tag = ("mm", "mm", "st", "st", "o", "o")[gi]
                        tiles[(t, half)] = pool.tile(
                            [128, 512], F32, tag=tag, name=f"vps{b}_{t}_{half}"
                        )
                    for c in range(C):
                        for t, half in group:
                            nc.tensor.matmul(
                                tiles[(t, half)],
                                lhsT=xts[b][:, c, t * 128 : (t + 1) * 128],
                                rhs=wv_sb[:, c, half * 512 : (half + 1) * 512],
                                start=(c == 0),
                                stop=(c == C - 1),
                            )
                    for t, half in group:
                        nc.vector.tensor_copy(
                            vaugs[b][t][:, half * 8 : (half + 1) * 8, 0:DH],
                            tiles[(t, half)].rearrange("p (h e) -> p h e", h=8),
                        )

            # ---- software-pipelined (batch, pair) job loop:
            #   ST(i) -> QK-proj(i+1) -> PV(i) ----
            jobs = [(b, pair) for b in range(B_LOC) for pair in range(H // 2)]
            osbs = {
                b: [
                    opool.tile([128, D], FP16, tag="osb", name=f"osb{b}_{sc}")
                    for sc in range(TC)
                ]
                for b in range(B_LOC)
            }

            def qk_proj(b, pair):
                qt = qkpool.tile([128, S], FP16, tag="qt", name=f"qt{b}_{pair}")
                kt = qkpool.tile([128, S], FP16, tag="kt", name=f"kt{b}_{pair}")
                for w_sb, dst in ((wq_sb, qt), (wk_sb, kt)):
                    ps = psmm.tile([128, 512], F32, tag="mm", name=f"qkps{b}_{pair}")
                    for c in range(C):
                        nc.tensor.matmul(
                            ps,
                            lhsT=w_sb[:, c, pair * 128 : (pair + 1) * 128],
                            rhs=xts[b][:, c, :],
                            start=(c == 0),
                            stop=(c == C - 1),
                        )
                    nc.vector.tensor_copy(dst, ps)
                return qt, kt

            def normalize(job_ot, job_b, job_pair, split_halves=False):
                """DVE normalize for a finished job + output flush. Runs one
                job late so the DVE never stalls on the XBAR transpose."""
                osb = osbs[job_b]
                halves = ((0, 2),) if not split_halves else ((0, 1), (1, 2))
                for hlo, hhi in halves:
                    nh = hhi - hlo
                    recip = rpool.tile([128, 2, TC, 1], F32, tag="r")
                    nc.vector.reciprocal(
                        recip[:, hlo:hhi].rearrange("p h s x -> p (h s) x"),
                        job_ot[:, hlo:hhi].rearrange("p h s q -> p (h s) q")[
                            :, :, DH : DH + 1
                        ],
                    )
                    for sc in range(TC):
                        nc.vector.scalar_tensor_tensor(
                            osb[sc][
                                :,
                                (job_pair * 2 + hlo) * DH : (job_pair * 2 + hhi) * DH,
                            ].rearrange("p (h e) -> p h e", h=nh),
                            job_ot[:, hlo:hhi, sc, 0:DH],
                            0.0,
                            recip[:, hlo:hhi, sc, :].to_broadcast([128, nh, DH]),
                            mybir.AluOpType.bypass,
                            mybir.AluOpType.mult,
                        )
                # output flush once a batch's last pair is normalized; the
                # final flush may split across both HWDGE queues (no ScalarE
                # work left to block), everything else stays on sync
                flush = []
                final = job_b == B_LOC - 1
                if final and job_pair == H // 4 - 1:
                    flush = [(0, D // 2)]
                elif final and job_pair == H // 2 - 1:
                    flush = [(D // 2, D)]
                elif job_pair == H // 2 - 1:
                    flush = [(0, D)]
                # flushes ride the GpSimd SWDGE (or the idle scalar queue at
                # the very end) so they never block the sync FIFO that feeds
                # the XBAR transposes
                very_last = final and job_pair == H // 2 - 1
                for lo, hi in flush:
                    for sc in range(TC):
                        eng = nc.scalar if (very_last and sc % 2 == 1) else nc.gpsimd
                        eng.dma_start(
                            out=out_d[job_b, sc * 128 : (sc + 1) * 128, lo:hi],
                            in_=osb[sc][:, lo:hi],
                        )

            qt, kt = qk_proj(*jobs[0])
            pending = None
            for ji, (b, pair) in enumerate(jobs):
                vaug = vaugs[b]
                osb = osbs[b]

                # ST matmuls: the two K=64 halves (base partitions 0/64)
                # row-pack; both land in one 2-bank psum tile so a single
                # [128,1024] exp covers both
                p_tiles = {}
                for t in range(TC):
                    ps = stp.tile([128, 2, 512], F32, tag="st", name=f"st{b}_{pair}")
                    for half in range(2):
                        lo, hi = 64 * half, 64 * (half + 1)
                        nc.tensor.matmul(
                            ps[:, half, :],
                            lhsT=kt[lo:hi, t * 128 : (t + 1) * 128],
                            rhs=qt[lo:hi, :],
                            start=True,
                            stop=True,
                        )
                    pt = ppool.tile([128, 2, 512], FP16, tag="p", name=f"p{b}_{pair}")
                    nc.scalar.activation(
                        pt.rearrange("p a b -> p (a b)"),
                        ps.rearrange("p a b -> p (a b)"),
                        mybir.ActivationFunctionType.Exp,
                        scale=float(SCALE),
                        bias=exp_bias[:, :],
                    )
                    for half in range(2):
                        p_tiles[(half, t)] = pt[:, half, :]

                # next job's projections fill the exp latency on the PE
                if ji + 1 < len(jobs):
                    nqt, nkt = qk_proj(*jobs[ji + 1])

                # PV: interleave the two half-chains so each LDW hides under
                # the other chain's streaming matmul
                oaugs = {}
                for half in range(2):
                    oaugs[half] = psout.tile(
                        [AUG, 512], F32, tag="o", name=f"oaug{b}_{pair}_{half}"
                    )
                for t in range(TC):
                    for half in range(2):
                        nc.tensor.matmul(
                            oaugs[half],
                            lhsT=vaug[t][:, pair * 2 + half, :],
                            rhs=p_tiles[(half, t)],
                            start=(t == 0),
                            stop=(t == TC - 1),
                        )
                # scaled fp16 copies (1/16 cancels in num/denom); split
                # between ScalarE and DVE to balance engine load
                oaug_sb = ovpool.tile([AUG, 2, 512], FP16, tag="oaug")
                nc.scalar.mul(oaug_sb[:, 0, :], oaugs[0], OSCALE)
                nc.vector.tensor_scalar_mul(oaug_sb[:, 1, :], oaugs[1], OSCALE)
                # XBAR transpose(s): [80, n*512] -> [128, (h sc), 80]. The
                # last job splits per half so normalize overlaps the second
                # transfer (shorter kernel tail)
                ot = otpool.tile([128, 2, TC, AUG], FP16, tag="ot")
                last = ji == len(jobs) - 1
                if last:
                    for half in range(2):
                        nc.sync.dma_start_transpose(
                            out=ot[:, half],
                            in_=oaug_sb[:, half, :],
                        )
                else:
                    nc.sync.dma_start_transpose(
                        out=ot.rearrange("p h s q -> p (h s) q"),
                        in_=oaug_sb.rearrange("q h s -> q (h s)"),
                    )
                # normalize the PREVIOUS job now: its transpose finished long
                # ago, so the strict-FIFO DVE won't stall and delay the next
                # job's qt/kt casts (which gate the PE)
                if pending is not None:
                    normalize(*pending)
                pending = (ot, b, pair)
                qt, kt = (nqt, nkt) if ji + 1 < len(jobs) else (None, None)

            normalize(*pending, split_halves=True)

    legalize_waits(nc)
    return nc


def _prep_inputs(x, Wq, Wk, Wv):
    x = np.ascontiguousarray(np.asarray(x, dtype=np.float32))
    # x [B, S, D] -> per-core partition-major xT [B_LOC, 2, 128, 4*S]:
    # element [b, hf, p, c'*S + s] = x[b, s, (4*hf+c')*128 + p]
    xt = x.reshape(N_CORES, B_LOC, S, 2, 4, 128).transpose(0, 1, 3, 5, 4, 2)
    xt = np.ascontiguousarray(xt).reshape(N_CORES, B_LOC, 2, 128, 4 * S)
    xt = xt.astype(np.float16)
    wp = []
    for W in (Wq, Wk, Wv):
        W = np.asarray(W, dtype=np.float32)
        # [H, D, DH] -> [D, H*DH] (d-major) -> [C, 128, H*DH]
        wp.append(
            np.ascontiguousarray(W.transpose(1, 0, 2))
            .reshape(C, 128, H * DH)
            .astype(np.float16)
        )
    return xt, wp[0], wp[1], wp[2]


_PROGRAM = None


def _get_program():
    global _PROGRAM
    if _PROGRAM is None:
        _PROGRAM = build_program()
    return _PROGRAM


def run(x, Wq, Wk, Wv, trace=False, nc=None):
    xt, wq_p, wk_p, wv_p = _prep_inputs(x, Wq, Wk, Wv)
    if nc is None:
        nc = _get_program()
    in_maps = [
        {"xt": xt[i], "wq": wq_p, "wk": wk_p, "wv": wv_p} for i in range(N_CORES)
    ]
    res = run_bass_kernel_spmd(nc, in_maps, list(range(N_CORES)), trace=trace)
    out = np.concatenate([res.results[i]["out"] for i in range(N_CORES)], axis=0)
    return out.astype(np.float32), res


def kernel(x, Wq, Wk, Wv):
    out, _ = run(x, Wq, Wk, Wv, trace=False)
    return out


# revision 12
# speedup vs baseline: 1.0377x; 1.0377x over previous
"""Multi-head attention Trainium2 kernel (Bass/Tile, SPMD over 8 cores).

fp16 compute variant: matmul operands in fp16 (2-byte stream rate = 2x fp32),
fp32 PSUM accumulation, fp32 normalization. Rel err vs fp32 reference ~1e-3.

Sharding: data parallel over batch. Core i computes batches [2i, 2i+2).

Structure per core:
  - Host pre-transposes x -> xT [d, s] partition-major and weights ->
    [d, h*e]; 4KB-line DMAs, split across both HWDGE queues (sync+scalar).
  - v projections for BOTH batches run first, chunk-major over 6 PSUM
    tiles so the PE ramps with DMA arrival instead of stalling per tile.
  - Job pipeline over (batch, pair): ST(i) -> QK-proj(i+1) -> PV(i); the
    next pair's projections fill the exp() latency between scores and PV.
  - Scores transposed ST[t,s]; exp on ScalarE with bias=-5 (fp16 headroom);
    the two heads of a pair sit at base partitions 0/64 so their K=64
    matmuls row-pack concurrently on the PE.
  - PV with V_aug stationary (ones cols 64:66, zero pad to 80 ->
    denominators ride along); the two half-chains interleave so LDW hides.
  - One XBAR DMA transpose per pair [80, 2x512] -> [128, (h s), 80]; one
    reciprocal per pair; fused scalar_tensor_tensor normalize per s-chunk
    (both heads at once, denominator broadcast via stride-0 AP).
  - Output fp16 on device; host casts to fp32.
"""

import numpy as np

import concourse.bass as bass
import concourse.mybir as mybir
import concourse.tile as tile
from concourse.bass_utils import run_bass_kernel_spmd

B, S, D, H, DH = 16, 512, 1024, 16, 64
N_CORES = 8
B_LOC = B // N_CORES  # 2 batches per core
C = D // 128  # 8 contraction chunks over d
TC = S // 128  # 4 chunks over s/t
AUG = 80  # DH + 2 ones cols + pad to 16-row multiple for the XBAR transpose
F32 = mybir.dt.float32
FP16 = mybir.dt.float16
SCALE = 1.0 / np.sqrt(np.float32(D))
EXP_BIAS = -5.0  # exp(logit-5): keeps P in fp16 range; cancels in normalize
OSCALE = 1.0 / 16.0  # pre-scale before fp16 transpose; cancels in normalize


def legalize_waits(nc, cap=1):
    """This walrus build supports at most `cap` sync-wait commands per
    instruction; hoist excess waits onto preceding same-engine NoOps."""
    n_split = 0
    for f in nc.m.functions:
        for blk in f.blocks:
            new_insts = []
            for inst in blk.instructions:
                si = getattr(inst, "sync_info", None)
                waits = list(si.on_wait) if si is not None and si.on_wait else []
                if len(waits) > cap:
                    keep, rest = waits[:cap], waits[cap:]
                    while rest:
                        chunk, rest = rest[:cap], rest[cap:]
                        nop = mybir.InstNoOp(
                            name=f"I-waitsplit-{nc.next_id()}", ins=[], outs=[]
                        )
                        nop.engine = inst.engine
                        nop.sync_info = mybir.SyncInfo(on_wait=chunk, on_update=[])
                        nc.register_instruction(nop, overwrite=True)
                        new_insts.append(nop)
                        n_split += 1
                    si.on_wait = keep
                new_insts.append(inst)
            blk.instructions[:] = new_insts
    return n_split


def build_program():
    nc = bass.Bass()
    # xt: partition-major per (batch, half): [B_LOC, 2, 128, 4*S] so each DMA
    # moves 4KB contiguous per partition.
    xt_d = nc.declare_dram_parameter("xt", [B_LOC, 2, 128, 4 * S], FP16, isOutput=False)
    wq_d = nc.declare_dram_parameter("wq", [C, 128, D], FP16, isOutput=False)
    wk_d = nc.declare_dram_parameter("wk", [C, 128, D], FP16, isOutput=False)
    wv_d = nc.declare_dram_parameter("wv", [C, 128, D], FP16, isOutput=False)
    out_d = nc.declare_dram_parameter("out", [B_LOC, S, D], FP16, isOutput=True)

    with tile.TileContext(nc) as tc:
        with (
            tc.tile_pool(name="wpool", bufs=1) as wpool,
            tc.tile_pool(name="xpool", bufs=1) as xpool,
            tc.tile_pool(name="vpool", bufs=4) as vpool,
            tc.tile_pool(name="qkpool", bufs=4) as qkpool,
            tc.tile_pool(name="ppool", bufs=10) as ppool,
            tc.tile_pool(name="opool", bufs=8) as opool,
            tc.tile_pool(name="rpool", bufs=4) as rpool,
            tc.tile_pool(name="ovpool", bufs=4) as ovpool,
            tc.tile_pool(name="otpool", bufs=4) as otpool,
            tc.tile_pool(name="psmm", bufs=2, space="PSUM") as psmm,
            tc.tile_pool(name="stp", bufs=2, space="PSUM") as stp,
            tc.tile_pool(name="psout", bufs=2, space="PSUM") as psout,
        ):
            exp_bias = ovpool.tile([128, 1], F32, tag="expbias", bufs=1)
            nc.vector.memset(exp_bias, EXP_BIAS)

            # ---- HAM warmup: ~4us of throwaway matmuls while the input DMAs
            # are in flight, so the PE clock gate is at 8/8 (2.4 GHz) when
            # real work arrives instead of ramping through it ----
            warm = ovpool.tile([128, 512], FP16, tag="warm", bufs=1)
            nc.vector.memset(warm, 0.0)
            wps = psmm.tile([128, 512], F32, tag="mm", name="warmps")
            for k in range(18):
                nc.tensor.matmul(
                    wps,
                    lhsT=warm[:, 0:128],
                    rhs=warm,
                    start=(k == 0),
                    stop=(k == 17),
                )

            # ---- input DMAs: xt + wq on the scalar HWDGE queue, wv + wk on
            # sync, so the two transfer streams run in parallel ----
            xts = [
                xpool.tile([128, C, S], FP16, tag=f"xt{b}", name=f"xt{b}")
                for b in range(B_LOC)
            ]
            wq_sb = wpool.tile([128, C, D], FP16, tag="wq")
            wk_sb = wpool.tile([128, C, D], FP16, tag="wk")
            wv_sb = wpool.tile([128, C, D], FP16, tag="wv")
            # b0 chunk-at-a-time so the first matmul can start early; b1 in
            # two big 4KB-line transfers
            for c in range(C):
                nc.scalar.dma_start(
                    out=xts[0][:, c, :],
                    in_=xt_d[0, c // 4, :, (c % 4) * S : (c % 4 + 1) * S],
                )
            for hf in range(2):
                nc.scalar.dma_start(
                    out=xts[1][:, 4 * hf : 4 * hf + 4, :].rearrange(
                        "p c s -> p (c s)"
                    ),
                    in_=xt_d[1, hf],
                )
            for c in range(C):
                nc.sync.dma_start(out=wv_sb[:, c, :], in_=wv_d[c])
            for g in range(2):
                nc.scalar.dma_start(
                    out=wq_sb[:, 4 * g : 4 * g + 4, :],
                    in_=wq_d[4 * g : 4 * g + 4].rearrange("c p d -> p c d"),
                )
                nc.sync.dma_start(
                    out=wk_sb[:, 4 * g : 4 * g + 4, :],
                    in_=wk_d[4 * g : 4 * g + 4].rearrange("c p d -> p c d"),
                )

            # ---- v projections for BOTH batches, chunk-major so each
            # arriving (xt, wv) chunk unlocks a burst of matmuls ----
            # V_aug layout [128(t), h, 64(e) + ones(2) + zero pad(14)]
            vaugs = {}
            for b in range(B_LOC):
                vaugs[b] = [
                    vpool.tile(
                        [128, H, AUG], FP16, tag=f"vaug{b}", name=f"vaug{b}_{t}"
                    )
                    for t in range(TC)
                ]
                for t in range(TC):
                    nc.vector.memset(vaugs[b][t][:, :, DH : DH + 2], 1.0)
                    nc.vector.memset(vaugs[b][t][:, :, DH + 2 : AUG], 0.0)
            for b in range(B_LOC):
                jobs = [(t, half) for t in range(TC) for half in range(2)]
                # groups of 6 and 2, borrowing stp/psout slots during this phase
                for group in (jobs[:6], jobs[6:]):
                    tiles = {}
                    for gi, (t, half) in enumerate(group):
                        pool = (psmm, psmm, stp, stp, psout, psout)[gi]
                        tag = ("mm", "mm", "st", "st", "o", "o")[gi]
                        tiles[(t, half)] = pool.tile(
                            [128, 512], F32, tag=tag, name=f"vps{b}_{t}_{half}"
                        )
                    for c in range(C):
                        for t, half in group:
                            nc.tensor.matmul(
                                tiles[(t, half)],
                                lhsT=xts[b][:, c, t * 128 : (t + 1) * 128],
                                rhs=wv_sb[:, c, half * 512 : (half + 1) * 512],
                                start=(c == 0),
                                stop=(c == C - 1),
                            )
                    for t, half in group:
                        nc.vector.tensor_copy(
                            vaugs[b][t][:, half * 8 : (half + 1) * 8, 0:DH],
                            tiles[(t, half)].rearrange("p (h e) -> p h e", h=8),
                        )

            # ---- software-pipelined (batch, pair) job loop:
            #   ST(i) -> QK-proj(i+1) -> PV(i) ----
            jobs = [(b, pair) for b in range(B_LOC) for pair in range(H // 2)]
            osbs = {
                b: [
                    opool.tile([128, D], FP16, tag="osb", name=f"osb{b}_{sc}")
                    for sc in range(TC)
                ]
                for b in range(B_LOC)
            }

            def qk_proj(b, pair):
                qt = qkpool.tile([128, S], FP16, tag="qt", name=f"qt{b}_{pair}")
                kt = qkpool.tile([128, S], FP16, tag="kt", name=f"kt{b}_{pair}")
                for w_sb, dst in ((wq_sb, qt), (wk_sb, kt)):
                    ps = psmm.tile([128, 512], F32, tag="mm", name=f"qkps{b}_{pair}")
                    for c in range(C):
                        nc.tensor.matmul(
                            ps,
                            lhsT=w_sb[:, c, pair * 128 : (pair + 1) * 128],
                            rhs=xts[b][:, c, :],
                            start=(c == 0),
                            stop=(c == C - 1),
                        )
                    nc.vector.tensor_copy(dst, ps)
                return qt, kt

            def normalize(job_ot, job_b, job_pair, split_halves=False):
                """DVE normalize for a finished job + output flush. Runs one
                job late so the DVE never stalls on the XBAR transpose."""
                osb = osbs[job_b]
                halves = ((0, 2),) if not split_halves else ((0, 1), (1, 2))
                for hlo, hhi in halves:
                    nh = hhi - hlo
                    recip = rpool.tile([128, 2, TC, 1], F32, tag="r")
                    nc.vector.reciprocal(
                        recip[:, hlo:hhi].rearrange("p h s x -> p (h s) x"),
                        job_ot[:, hlo:hhi].rearrange("p h s q -> p (h s) q")[
                            :, :, DH : DH + 1
                        ],
                    )
                    for sc in range(TC):
                        nc.vector.scalar_tensor_tensor(
                            osb[sc][
                                :,
                                (job_pair * 2 + hlo) * DH : (job_pair * 2 + hhi) * DH,
                            ].rearrange("p (h e) -> p h e", h=nh),
                            job_ot[:, hlo:hhi, sc, 0:DH],
                            0.0,
                            recip[:, hlo:hhi, sc, :].to_broadcast([128, nh, DH]),
                            mybir.AluOpType.bypass,
                            mybir.AluOpType.mult,
                        )
                # output flush once a batch's last pair is normalized; the
                # final flush may split across both HWDGE queues (no ScalarE
                # work left to block), everything else stays on sync
                flush = []
                final = job_b == B_LOC - 1
                if final and job_pair == H // 4 - 1:
                    flush = [(0, D // 2)]
                elif final and job_pair == 3 * H // 8 - 1:
                    flush = [(D // 2, 3 * D // 4)]
                elif final and job_pair == H // 2 - 1:
                    flush = [(3 * D // 4, D)]
                elif job_pair == H // 2 - 1:
                    flush = [(0, D)]
                # flushes ride the GpSimd SWDGE (or the idle scalar queue at
                # the very end) so they never block the sync FIFO that feeds
                # the XBAR transposes
                very_last = final and job_pair == H // 2 - 1
                for lo, hi in flush:
                    for sc in range(TC):
                        eng = nc.scalar if (very_last and sc % 2 == 1) else nc.gpsimd
                        eng.dma_start(
                            out=out_d[job_b, sc * 128 : (sc + 1) * 128, lo:hi],
                            in_=osb[sc][:, lo:hi],
                        )

            qt, kt = qk_proj(*jobs[0])
            pending = None
            for ji, (b, pair) in enumerate(jobs):
                vaug = vaugs[b]
                osb = osbs[b]

                # ST matmuls: the two K=64 halves (base partitions 0/64)
                # row-pack; both land in one 2-bank psum tile so a single
                # [128,1024] exp covers both
                p_tiles = {}
                for t in range(TC):
                    ps = stp.tile([128, 2, 512], F32, tag="st", name=f"st{b}_{pair}")
                    for half in range(2):
                        lo, hi = 64 * half, 64 * (half + 1)
                        nc.tensor.matmul(
                            ps[:, half, :],
                            lhsT=kt[lo:hi, t * 128 : (t + 1) * 128],
                            rhs=qt[lo:hi, :],
                            start=True,
                            stop=True,
                        )
                    pt = ppool.tile([128, 2, 512], FP16, tag="p", name=f"p{b}_{pair}")
                    nc.scalar.activation(
                        pt.rearrange("p a b -> p (a b)"),
                        ps.rearrange("p a b -> p (a b)"),
                        mybir.ActivationFunctionType.Exp,
                        scale=float(SCALE),
                        bias=exp_bias[:, :],
                    )
                    for half in range(2):
                        p_tiles[(half, t)] = pt[:, half, :]

                # next job's projections fill the exp latency on the PE
                if ji + 1 < len(jobs):
                    nqt, nkt = qk_proj(*jobs[ji + 1])

                # PV: interleave the two half-chains so each LDW hides under
                # the other chain's streaming matmul
                oaugs = {}
                for half in range(2):
                    oaugs[half] = psout.tile(
                        [AUG, 512], F32, tag="o", name=f"oaug{b}_{pair}_{half}"
                    )
                for t in range(TC):
                    for half in range(2):
                        nc.tensor.matmul(
                            oaugs[half],
                            lhsT=vaug[t][:, pair * 2 + half, :],
                            rhs=p_tiles[(half, t)],
                            start=(t == 0),
                            stop=(t == TC - 1),
                        )
                # scaled fp16 copies (1/16 cancels in num/denom); both on
                # ScalarE: on the DVE the h1 copy queues behind next-job
                # casts, delaying the transpose -> normalize -> PE chain
                oaug_sb = ovpool.tile([AUG, 2, 512], FP16, tag="oaug")
                nc.scalar.mul(oaug_sb[:, 0, :], oaugs[0], OSCALE)
                nc.scalar.mul(oaug_sb[:, 1, :], oaugs[1], OSCALE)
                # XBAR transpose(s): [80, n*512] -> [128, (h sc), 80]. The
                # last job splits per half so normalize overlaps the second
                # transfer (shorter kernel tail)
                ot = otpool.tile([128, 2, TC, AUG], FP16, tag="ot")
                last = ji == len(jobs) - 1
                if last:
                    for half in range(2):
                        nc.sync.dma_start_transpose(
                            out=ot[:, half],
                            in_=oaug_sb[:, half, :],
                        )
                else:
                    nc.sync.dma_start_transpose(
                        out=ot.rearrange("p h s q -> p (h s) q"),
                        in_=oaug_sb.rearrange("q h s -> q (h s)"),
                    )
                # normalize the PREVIOUS job now: its transpose finished long
                # ago, so the strict-FIFO DVE won't stall and delay the next
                # job's qt/kt casts (which gate the PE)
                if pending is not None:
                    normalize(*pending)
                pending = (ot, b, pair)
                qt, kt = (nqt, nkt) if ji + 1 < len(jobs) else (None, None)

            normalize(*pending, split_halves=True)

    legalize_waits(nc)
    return nc


def _prep_inputs(x, Wq, Wk, Wv):
    x = np.ascontiguousarray(np.asarray(x, dtype=np.float32))
    # x [B, S, D] -> per-core partition-major xT [B_LOC, 2, 128, 4*S]:
    # element [b, hf, p, c'*S + s] = x[b, s, (4*hf+c')*128 + p]
    xt = x.reshape(N_CORES, B_LOC, S, 2, 4, 128).transpose(0, 1, 3, 5, 4, 2)
    xt = np.ascontiguousarray(xt).reshape(N_CORES, B_LOC, 2, 128, 4 * S)
    xt = xt.astype(np.float16)
    wp = []
    for W in (Wq, Wk, Wv):
        W = np.asarray(W, dtype=np.float32)
        # [H, D, DH] -> [D, H*DH] (d-major) -> [C, 128, H*DH]
        wp.append(
            np.ascontiguousarray(W.transpose(1, 0, 2))
            .reshape(C, 128, H * DH)
            .astype(np.float16)
        )
    return xt, wp[0], wp[1], wp[2]


_PROGRAM = None


def _get_program():
    global _PROGRAM
    if _PROGRAM is None:
        _PROGRAM = build_program()
    return _PROGRAM


def run(x, Wq, Wk, Wv, trace=False, nc=None):
    xt, wq_p, wk_p, wv_p = _prep_inputs(x, Wq, Wk, Wv)
    if nc is None:
        nc = _get_program()
    in_maps = [
        {"xt": xt[i], "wq": wq_p, "wk": wk_p, "wv": wv_p} for i in range(N_CORES)
    ]
    res = run_bass_kernel_spmd(nc, in_maps, list(range(N_CORES)), trace=trace)
    out = np.concatenate([res.results[i]["out"] for i in range(N_CORES)], axis=0)
    return out.astype(np.float32), res


def kernel(x, Wq, Wk, Wv):
    out, _ = run(x, Wq, Wk, Wv, trace=False)
    return out


# revision 16
# speedup vs baseline: 1.0448x; 1.0069x over previous
"""Multi-head attention Trainium2 kernel (Bass/Tile, SPMD over 8 cores).

fp16 compute variant: matmul operands in fp16 (2-byte stream rate = 2x fp32),
fp32 PSUM accumulation, fp32 normalization. Rel err vs fp32 reference ~1e-3.

Sharding: data parallel over batch. Core i computes batches [2i, 2i+2).

Structure per core:
  - Host pre-transposes x -> xT [d, s] partition-major and weights ->
    [d, h*e]; 4KB-line DMAs, split across both HWDGE queues (sync+scalar).
  - v projections for BOTH batches run first, chunk-major over 6 PSUM
    tiles so the PE ramps with DMA arrival instead of stalling per tile.
  - Job pipeline over (batch, pair): ST(i) -> QK-proj(i+1) -> PV(i); the
    next pair's projections fill the exp() latency between scores and PV.
  - Scores transposed ST[t,s]; exp on ScalarE with bias=-5 (fp16 headroom);
    the two heads of a pair sit at base partitions 0/64 so their K=64
    matmuls row-pack concurrently on the PE.
  - PV with V_aug stationary (ones cols 64:66, zero pad to 80 ->
    denominators ride along); the two half-chains interleave so LDW hides.
  - One XBAR DMA transpose per pair [80, 2x512] -> [128, (h s), 80]; one
    reciprocal per pair; fused scalar_tensor_tensor normalize per s-chunk
    (both heads at once, denominator broadcast via stride-0 AP).
  - Output fp16 on device; host casts to fp32.
"""

import numpy as np

import concourse.bass as bass
import concourse.mybir as mybir
import concourse.tile as tile
from concourse.bass_utils import run_bass_kernel_spmd

B, S, D, H, DH = 16, 512, 1024, 16, 64
N_CORES = 8
B_LOC = B // N_CORES  # 2 batches per core
C = D // 128  # 8 contraction chunks over d
TC = S // 128  # 4 chunks over s/t
AUG = 80  # DH + 2 ones cols + pad to 16-row multiple for the XBAR transpose
F32 = mybir.dt.float32
FP16 = mybir.dt.float16
SCALE = 1.0 / np.sqrt(np.float32(D))
EXP_BIAS = -5.0  # exp(logit-5): keeps P in fp16 range; cancels in normalize
OSCALE = 1.0 / 16.0  # pre-scale before fp16 transpose; cancels in normalize


def legalize_waits(nc, cap=1):
    """This walrus build supports at most `cap` sync-wait commands per
    instruction; hoist excess waits onto preceding same-engine NoOps."""
    n_split = 0
    for f in nc.m.functions:
        for blk in f.blocks:
            new_insts = []
            for inst in blk.instructions:
                si = getattr(inst, "sync_info", None)
                waits = list(si.on_wait) if si is not None and si.on_wait else []
                if len(waits) > cap:
                    keep, rest = waits[:cap], waits[cap:]
                    while rest:
                        chunk, rest = rest[:cap], rest[cap:]
                        nop = mybir.InstNoOp(
                            name=f"I-waitsplit-{nc.next_id()}", ins=[], outs=[]
                        )
                        nop.engine = inst.engine
                        nop.sync_info = mybir.SyncInfo(on_wait=chunk, on_update=[])
                        nc.register_instruction(nop, overwrite=True)
                        new_insts.append(nop)
                        n_split += 1
                    si.on_wait = keep
                new_insts.append(inst)
            blk.instructions[:] = new_insts
    return n_split


def build_program():
    nc = bass.Bass()
    # xt: partition-major per (batch, half): [B_LOC, 2, 128, 4*S] so each DMA
    # moves 4KB contiguous per partition.
    xt_d = nc.declare_dram_parameter("xt", [B_LOC, 2, 128, 4 * S], FP16, isOutput=False)
    wq_d = nc.declare_dram_parameter("wq", [C, 128, D], FP16, isOutput=False)
    wk_d = nc.declare_dram_parameter("wk", [C, 128, D], FP16, isOutput=False)
    wv_d = nc.declare_dram_parameter("wv", [C, 128, D], FP16, isOutput=False)
    out_d = nc.declare_dram_parameter("out", [B_LOC, S, D], FP16, isOutput=True)

    with tile.TileContext(nc) as tc:
        with (
            tc.tile_pool(name="wpool", bufs=1) as wpool,
            tc.tile_pool(name="xpool", bufs=1) as xpool,
            tc.tile_pool(name="vpool", bufs=4) as vpool,
            tc.tile_pool(name="qkpool", bufs=6) as qkpool,
            tc.tile_pool(name="ppool", bufs=10) as ppool,
            tc.tile_pool(name="opool", bufs=8) as opool,
            tc.tile_pool(name="rpool", bufs=4) as rpool,
            tc.tile_pool(name="ovpool", bufs=4) as ovpool,
            tc.tile_pool(name="otpool", bufs=4) as otpool,
            tc.tile_pool(name="psmm", bufs=2, space="PSUM") as psmm,
            tc.tile_pool(name="stp", bufs=2, space="PSUM") as stp,
            tc.tile_pool(name="psout", bufs=2, space="PSUM") as psout,
        ):
            exp_bias = ovpool.tile([128, 1], F32, tag="expbias", bufs=1)
            nc.vector.memset(exp_bias, EXP_BIAS)

            # ---- HAM warmup: ~4us of throwaway matmuls while the input DMAs
            # are in flight, so the PE clock gate is at 8/8 (2.4 GHz) when
            # real work arrives instead of ramping through it ----
            warm = ovpool.tile([128, 512], FP16, tag="warm", bufs=1)
            nc.vector.memset(warm, 0.0)
            wps = psmm.tile([128, 512], F32, tag="mm", name="warmps")
            for k in range(18):
                nc.tensor.matmul(
                    wps,
                    lhsT=warm[:, 0:128],
                    rhs=warm,
                    start=(k == 0),
                    stop=(k == 17),
                )

            # ---- input DMAs: xt + wq on the scalar HWDGE queue, wv + wk on
            # sync, so the two transfer streams run in parallel ----
            xts = [
                xpool.tile([128, C, S], FP16, tag=f"xt{b}", name=f"xt{b}")
                for b in range(B_LOC)
            ]
            wq_sb = wpool.tile([128, C, D], FP16, tag="wq")
            wk_sb = wpool.tile([128, C, D], FP16, tag="wk")
            wv_sb = wpool.tile([128, C, D], FP16, tag="wv")
            # b0 chunk-at-a-time so the first matmul can start early; b1 in
            # two big 4KB-line transfers
            for c in range(C):
                nc.scalar.dma_start(
                    out=xts[0][:, c, :],
                    in_=xt_d[0, c // 4, :, (c % 4) * S : (c % 4 + 1) * S],
                )
            for hf in range(2):
                nc.scalar.dma_start(
                    out=xts[1][:, 4 * hf : 4 * hf + 4, :].rearrange(
                        "p c s -> p (c s)"
                    ),
                    in_=xt_d[1, hf],
                )
            for c in range(C):
                nc.sync.dma_start(out=wv_sb[:, c, :], in_=wv_d[c])
            for g in range(2):
                nc.scalar.dma_start(
                    out=wq_sb[:, 4 * g : 4 * g + 4, :],
                    in_=wq_d[4 * g : 4 * g + 4].rearrange("c p d -> p c d"),
                )
                nc.sync.dma_start(
                    out=wk_sb[:, 4 * g : 4 * g + 4, :],
                    in_=wk_d[4 * g : 4 * g + 4].rearrange("c p d -> p c d"),
                )

            # ---- v projections for BOTH batches, chunk-major so each
            # arriving (xt, wv) chunk unlocks a burst of matmuls ----
            # V_aug layout [128(t), h, 64(e) + ones(2) + zero pad(14)]
            vaugs = {}
            for b in range(B_LOC):
                vaugs[b] = [
                    vpool.tile(
                        [128, H, AUG], FP16, tag=f"vaug{b}", name=f"vaug{b}_{t}"
                    )
                    for t in range(TC)
                ]
                for t in range(TC):
                    nc.vector.memset(vaugs[b][t][:, :, DH : DH + 2], 1.0)
                    nc.vector.memset(vaugs[b][t][:, :, DH + 2 : AUG], 0.0)
            for b in range(B_LOC):
                jobs = [(t, half) for t in range(TC) for half in range(2)]
                # groups of 6 and 2, borrowing stp/psout slots during this phase
                for group in (jobs[:6], jobs[6:]):
                    tiles = {}
                    for gi, (t, half) in enumerate(group):
                        pool = (psmm, psmm, stp, stp, psout, psout)[gi]
                        tag = ("mm", "mm", "st", "st", "o", "o")[gi]
                        tiles[(t, half)] = pool.tile(
                            [128, 512], F32, tag=tag, name=f"vps{b}_{t}_{half}"
                        )
                    for c in range(C):
                        for t, half in group:
                            nc.tensor.matmul(
                                tiles[(t, half)],
                                lhsT=xts[b][:, c, t * 128 : (t + 1) * 128],
                                rhs=wv_sb[:, c, half * 512 : (half + 1) * 512],
                                start=(c == 0),
                                stop=(c == C - 1),
                            )
                    for t, half in group:
                        nc.vector.tensor_copy(
                            vaugs[b][t][:, half * 8 : (half + 1) * 8, 0:DH],
                            tiles[(t, half)].rearrange("p (h e) -> p h e", h=8),
                        )

            # ---- software-pipelined (batch, pair) job loop:
            #   ST(i) -> QK-proj(i+1) -> PV(i) ----
            jobs = [(b, pair) for b in range(B_LOC) for pair in range(H // 2)]
            osbs = {
                b: [
                    opool.tile([128, D], FP16, tag="osb", name=f"osb{b}_{sc}")
                    for sc in range(TC)
                ]
                for b in range(B_LOC)
            }

            def qk_proj(b, pair):
                qt = qkpool.tile([128, S], FP16, tag="qt", name=f"qt{b}_{pair}")
                kt = qkpool.tile([128, S], FP16, tag="kt", name=f"kt{b}_{pair}")
                for w_sb, dst in ((wq_sb, qt), (wk_sb, kt)):
                    ps = psmm.tile([128, 512], F32, tag="mm", name=f"qkps{b}_{pair}")
                    for c in range(C):
                        nc.tensor.matmul(
                            ps,
                            lhsT=w_sb[:, c, pair * 128 : (pair + 1) * 128],
                            rhs=xts[b][:, c, :],
                            start=(c == 0),
                            stop=(c == C - 1),
                        )
                    nc.vector.tensor_copy(dst, ps)
                return qt, kt

            def normalize(job_ot, job_b, job_pair, split_halves=False):
                """DVE normalize for a finished job + output flush. Runs one
                job late so the DVE never stalls on the XBAR transpose."""
                osb = osbs[job_b]
                halves = ((0, 2),) if not split_halves else ((0, 1), (1, 2))
                for hlo, hhi in halves:
                    nh = hhi - hlo
                    recip = rpool.tile([128, 2, TC, 1], F32, tag="r")
                    nc.vector.reciprocal(
                        recip[:, hlo:hhi].rearrange("p h s x -> p (h s) x"),
                        job_ot[:, hlo:hhi].rearrange("p h s q -> p (h s) q")[
                            :, :, DH : DH + 1
                        ],
                    )
                    for sc in range(TC):
                        nc.vector.scalar_tensor_tensor(
                            osb[sc][
                                :,
                                (job_pair * 2 + hlo) * DH : (job_pair * 2 + hhi) * DH,
                            ].rearrange("p (h e) -> p h e", h=nh),
                            job_ot[:, hlo:hhi, sc, 0:DH],
                            0.0,
                            recip[:, hlo:hhi, sc, :].to_broadcast([128, nh, DH]),
                            mybir.AluOpType.bypass,
                            mybir.AluOpType.mult,
                        )
                # output flush once a batch's last pair is normalized; the
                # final flush may split across both HWDGE queues (no ScalarE
                # work left to block), everything else stays on sync
                flush = []
                final = job_b == B_LOC - 1
                if final and job_pair == H // 4 - 1:
                    flush = [(0, D // 2)]
                elif final and job_pair == 3 * H // 8 - 1:
                    flush = [(D // 2, 3 * D // 4)]
                elif final and job_pair == H // 2 - 1:
                    flush = [(3 * D // 4, D)]
                elif job_pair == H // 2 - 1:
                    flush = [(0, D)]
                # flushes ride the GpSimd SWDGE (or the idle scalar queue at
                # the very end) so they never block the sync FIFO that feeds
                # the XBAR transposes
                very_last = final and job_pair == H // 2 - 1
                for lo, hi in flush:
                    for sc in range(TC):
                        eng = nc.scalar if (very_last and sc % 2 == 1) else nc.gpsimd
                        eng.dma_start(
                            out=out_d[job_b, sc * 128 : (sc + 1) * 128, lo:hi],
                            in_=osb[sc][:, lo:hi],
                        )

            # project TWO jobs ahead: the qt/kt casts then have two job
            # periods of slack, so scheduler-ordered DVE work (normalize)
            # can never delay the ST matmuls that need them
            qk_queue = [qk_proj(*jobs[0]), qk_proj(*jobs[1])]
            pending = None
            for ji, (b, pair) in enumerate(jobs):
                vaug = vaugs[b]
                osb = osbs[b]
                qt, kt = qk_queue.pop(0)

                # ST matmuls: the two K=64 halves (base partitions 0/64)
                # row-pack; both land in one 2-bank psum tile so a single
                # [128,1024] exp covers both
                p_tiles = {}
                for t in range(TC):
                    ps = stp.tile([128, 2, 512], F32, tag="st", name=f"st{b}_{pair}")
                    for half in range(2):
                        lo, hi = 64 * half, 64 * (half + 1)
                        nc.tensor.matmul(
                            ps[:, half, :],
                            lhsT=kt[lo:hi, t * 128 : (t + 1) * 128],
                            rhs=qt[lo:hi, :],
                            start=True,
                            stop=True,
                        )
                    pt = ppool.tile([128, 2, 512], FP16, tag="p", name=f"p{b}_{pair}")
                    nc.scalar.activation(
                        pt.rearrange("p a b -> p (a b)"),
                        ps.rearrange("p a b -> p (a b)"),
                        mybir.ActivationFunctionType.Exp,
                        scale=float(SCALE),
                        bias=exp_bias[:, :],
                    )
                    for half in range(2):
                        p_tiles[(half, t)] = pt[:, half, :]

                # the job-after-next's projections fill the exp latency on
                # the PE
                if ji + 2 < len(jobs):
                    qk_queue.append(qk_proj(*jobs[ji + 2]))

                # PV: interleave the two half-chains so each LDW hides under
                # the other chain's streaming matmul
                oaugs = {}
                for half in range(2):
                    oaugs[half] = psout.tile(
                        [AUG, 512], F32, tag="o", name=f"oaug{b}_{pair}_{half}"
                    )
                for t in range(TC):
                    for half in range(2):
                        nc.tensor.matmul(
                            oaugs[half],
                            lhsT=vaug[t][:, pair * 2 + half, :],
                            rhs=p_tiles[(half, t)],
                            start=(t == 0),
                            stop=(t == TC - 1),
                        )
                # scaled fp16 copies (1/16 cancels in num/denom); both on
                # ScalarE: on the DVE the h1 copy queues behind next-job
                # casts, delaying the transpose -> normalize -> PE chain
                oaug_sb = ovpool.tile([AUG, 2, 512], FP16, tag="oaug")
                nc.scalar.mul(oaug_sb[:, 0, :], oaugs[0], OSCALE)
                nc.scalar.mul(oaug_sb[:, 1, :], oaugs[1], OSCALE)
                # XBAR transpose(s): [80, n*512] -> [128, (h sc), 80]. The
                # last job splits per half so normalize overlaps the second
                # transfer (shorter kernel tail)
                ot = otpool.tile([128, 2, TC, AUG], FP16, tag="ot")
                last = ji == len(jobs) - 1
                if last:
                    for half in range(2):
                        nc.sync.dma_start_transpose(
                            out=ot[:, half],
                            in_=oaug_sb[:, half, :],
                        )
                else:
                    nc.sync.dma_start_transpose(
                        out=ot.rearrange("p h s q -> p (h s) q"),
                        in_=oaug_sb.rearrange("q h s -> q (h s)"),
                    )
                # normalize the PREVIOUS job now: its transpose finished long
                # ago, so the strict-FIFO DVE won't stall and delay the next
                # job's qt/kt casts (which gate the PE)
                if pending is not None:
                    normalize(*pending)
                pending = (ot, b, pair)

            normalize(*pending, split_halves=True)

    legalize_waits(nc)
    return nc


def _prep_inputs(x, Wq, Wk, Wv):
    x = np.ascontiguousarray(np.asarray(x, dtype=np.float32))
    # x [B, S, D] -> per-core partition-major xT [B_LOC, 2, 128, 4*S]:
    # element [b, hf, p, c'*S + s] = x[b, s, (4*hf+c')*128 + p]
    xt = x.reshape(N_CORES, B_LOC, S, 2, 4, 128).transpose(0, 1, 3, 5, 4, 2)
    xt = np.ascontiguousarray(xt).reshape(N_CORES, B_LOC, 2, 128, 4 * S)
    xt = xt.astype(np.float16)
    wp = []
    for W in (Wq, Wk, Wv):
        W = np.asarray(W, dtype=np.float32)
        # [H, D, DH] -> [D, H*DH] (d-major) -> [C, 128, H*DH]
        wp.append(
            np.ascontiguousarray(W.transpose(1, 0, 2))
            .reshape(C, 128, H * DH)
            .astype(np.float16)
        )
    return xt, wp[0], wp[1], wp[2]


_PROGRAM = None


def _get_program():
    global _PROGRAM
    if _PROGRAM is None:
        _PROGRAM = build_program()
    return _PROGRAM


def run(x, Wq, Wk, Wv, trace=False, nc=None):
    xt, wq_p, wk_p, wv_p = _prep_inputs(x, Wq, Wk, Wv)
    if nc is None:
        nc = _get_program()
    in_maps = [
        {"xt": xt[i], "wq": wq_p, "wk": wk_p, "wv": wv_p} for i in range(N_CORES)
    ]
    res = run_bass_kernel_spmd(nc, in_maps, list(range(N_CORES)), trace=trace)
    out = np.concatenate([res.results[i]["out"] for i in range(N_CORES)], axis=0)
    return out.astype(np.float32), res


def kernel(x, Wq, Wk, Wv):
    out, _ = run(x, Wq, Wk, Wv, trace=False)
    return out
